# revision 1
# baseline (speedup 1.0000x reference)
"""Multi-head attention (B=2, S=2048, D=2048, H=16) on 8 Trainium2 cores.

Sharding: core = batch (2) x head-group (4 heads each). Tensor-parallel on
wq/wk/wv rows + wo columns; per-core partial outputs summed on host.

Device kernel (per core, all matmuls in float32r):
  phase 1: qT/kT (head_dim, seq) + v (seq, head_dim) projections, RoPE on q/k
  phase 2: scores^T -> exp -> denominator (ones-matmul) + attn@V, normalize
  phase 3: output projection partial (seq, dim)
"""

import sys

for _p in ("/opt/trn_rl_repo",):
    if _p not in sys.path:
        sys.path.insert(0, _p)

import numpy as np

import concourse.bass as bass
import concourse.tile as tile
from concourse import bacc, mybir
from concourse.bass_utils import run_bass_kernel_spmd

F32 = mybir.dt.float32
F32R = mybir.dt.float32r

DIM = 2048
N_HEADS = 16
HEAD_DIM = 128
BATCH = 2
SEQ = 2048
G_HEADS = 4          # heads per core
GM = G_HEADS * HEAD_DIM  # 512 output cols per core
DC = DIM // 128      # 16 contraction chunks
SC512 = SEQ // 512   # 4
SC128 = SEQ // 128   # 16
INV_SQRT_HD = float(1.0 / np.sqrt(HEAD_DIM))

# even<->odd partition swap within each 32-partition group
_SWAP_MASK = [i ^ 1 for i in range(32)]


def build(with_mask: bool):
    nc = bacc.Bacc("TRN2", target_bir_lowering=False, debug=False)

    xt_d = nc.dram_tensor("xt", [DC, 128, SEQ], F32R, kind="ExternalInput").ap()
    wq_d = nc.dram_tensor("wq", [DC, G_HEADS, 128, 128], F32R, kind="ExternalInput").ap()
    wk_d = nc.dram_tensor("wk", [DC, G_HEADS, 128, 128], F32R, kind="ExternalInput").ap()
    wv_d = nc.dram_tensor("wv", [DC, 128, GM], F32R, kind="ExternalInput").ap()
    wo_d = nc.dram_tensor("wo", [G_HEADS, 128, SEQ], F32R, kind="ExternalInput").ap()
    ce_d = nc.dram_tensor("ce", [128, SEQ], F32R, kind="ExternalInput").ap()
    s2_d = nc.dram_tensor("s2", [128, SEQ], F32R, kind="ExternalInput").ap()
    ones_d = nc.dram_tensor("ones", [128, 1], F32R, kind="ExternalInput").ap()
    mt_d = None
    if with_mask:
        mt_d = nc.dram_tensor("mt", [SC128, 128, SEQ], F32, kind="ExternalInput").ap()
    out_d = nc.dram_tensor("out", [SC128, 128, SEQ], F32, kind="ExternalOutput").ap()

    with tile.TileContext(nc) as tc:
        with (
            tc.tile_pool(name="persist", bufs=1) as persist,
            tc.tile_pool(name="consts", bufs=1) as consts,
        ):
            ones_t = consts.tile([128, 1], F32R, tag="ones")
            nc.sync.dma_start(ones_t[:], ones_d)
            # warm the ACT exp LUT early so phase 2 doesn't stall on it
            warm_t = consts.tile([128, 1], F32, tag="warm")
            nc.scalar.activation(
                out=warm_t[:], in_=ones_t[:],
                func=mybir.ActivationFunctionType.Exp,
            )

            q_t = [persist.tile([128, SEQ], F32R, tag=f"q{h}", name=f"q{h}") for h in range(G_HEADS)]
            k_t = [persist.tile([128, SEQ], F32R, tag=f"k{h}", name=f"k{h}") for h in range(G_HEADS)]
            v_t = [persist.tile([128, GM], F32R, tag=f"v{s}", name=f"v{s}") for s in range(SC128)]

            # ---------------- phase 1: projections + fused rope ----------------
            with (
                tc.tile_pool(name="rope_c", bufs=1) as rope_c,
                tc.tile_pool(name="xt", bufs=8) as xt_pool,
                tc.tile_pool(name="wqk", bufs=2) as wqk_pool,
                tc.tile_pool(name="wv", bufs=1) as wv_pool,
                tc.tile_pool(name="ps1", bufs=4, space="PSUM") as ps1,
                tc.tile_pool(name="rope_t", bufs=1) as rope_t,
            ):
                ce_t = rope_c.tile([128, SEQ], F32R, tag="ce")
                s2_t = rope_c.tile([128, SEQ], F32R, tag="s2")

                def rope(t, sl):
                    # sin-product on the otherwise-idle GpSimd engine
                    t1 = rope_t.tile([128, 512], F32, tag="t1", name="t1")
                    nc.gpsimd.tensor_mul(out=t1[:], in0=t[:, sl], in1=s2_t[:, sl])
                    t2 = rope_t.tile([128, 512], F32, tag="t2", name="t2")
                    nc.vector.stream_shuffle(t2[:], t1[:], _SWAP_MASK)
                    t3 = rope_t.tile([128, 512], F32, tag="t3", name="t3")
                    nc.vector.tensor_mul(out=t3[:], in0=t[:, sl], in1=ce_t[:, sl])
                    nc.vector.tensor_add(out=t[:, sl], in0=t3[:], in1=t2[:])

                for half in range(2):
                    dcs = list(range(half * 8, half * 8 + 8))
                    # first head's weights + first-half x tiles are what the
                    # very first matmul chain needs: pin them to the front of
                    # the scheduler's priority heap so no other dep-free DMA
                    # (wvt/ce/s2/...) gets hoisted ahead of them
                    prio = tc.high_priority() if half == 0 else None
                    if prio is not None:
                        prio.__enter__()
                    wt_first = wqk_pool.tile([128, 8, 128], F32R, tag="w", name="wt")
                    nc.sync.dma_start(
                        wt_first[:],
                        wq_d[dcs[0] : dcs[0] + 8, 0].rearrange("c p m -> p c m"),
                    )
                    # x tiles split into s-halves (separate tiles so the
                    # first chains unblock after half the data), alternating
                    # issue queues (SP / ACT) for parallel DMA
                    xtsA, xtsB = [], []
                    for qi, dc in enumerate(dcs):
                        xa = xt_pool.tile([128, 1024], F32R, tag="xa", name="xa")
                        eng = nc.sync if qi % 2 == 0 else nc.scalar
                        eng.dma_start(xa[:], xt_d[dc][:, 0:1024])
                        xtsA.append(xa)
                    if prio is not None:
                        prio.__exit__(None, None, None)
                    for qi, dc in enumerate(dcs):
                        xb = xt_pool.tile([128, 1024], F32R, tag="xb", name="xb")
                        eng = nc.sync if qi % 2 == 1 else nc.scalar
                        eng.dma_start(xb[:], xt_d[dc][:, 1024:2048])
                        xtsB.append(xb)

                    def xslice(i, sl_start, width):
                        # rhs slice [sl_start, sl_start+width) of logical xt[i]
                        if sl_start < 1024:
                            return xtsA[i][:, sl_start : sl_start + width]
                        return xtsB[i][:, sl_start - 1024 : sl_start - 1024 + width]
                    wvt = wv_pool.tile([128, 8, GM], F32R, tag="wv", name="wvt")
                    nc.scalar.dma_start(
                        wvt[:], wv_d[dcs[0] : dcs[0] + 8].rearrange("c p m -> p c m")
                    )
                    v_next = 0

                    def emit_v(n):
                        nonlocal v_next
                        for s in range(v_next, v_next + n):
                            ps = ps1.tile([128, GM], F32, tag="ps", name="ps")
                            for i in range(8):
                                nc.tensor.matmul(
                                    ps[:], xslice(i, s * 128, 128), wvt[:, i, :],
                                    start=(i == 0), stop=(i == 7),
                                )
                            if half == 0:
                                nc.vector.tensor_copy(out=v_t[s][:], in_=ps[:])
                            else:
                                nc.vector.tensor_add(
                                    out=v_t[s][:], in0=ps[:], in1=v_t[s][:]
                                )
                        v_next += n

                    # v-group placement: back-loaded in half 0 (wvt DMA queues
                    # behind the xt bulk); interleaved finely in half 1 so DVE
                    # rope work never outpaces PE for long
                    for h in range(G_HEADS):
                        for wi, (wd, dst) in enumerate(((wq_d, q_t[h]), (wk_d, k_t[h]))):
                            if h == 0 and wi == 0:
                                wt = wt_first
                            else:
                                wt = wqk_pool.tile([128, 8, 128], F32R, tag="w", name="wt")
                                nc.sync.dma_start(
                                    wt[:],
                                    wd[dcs[0] : dcs[0] + 8, h].rearrange("c p m -> p c m"),
                                )
                            for sc in range(SC512):
                                ps = ps1.tile([128, 512], F32, tag="ps", name="ps")
                                sl = bass.ts(sc, 512)
                                for i in range(8):
                                    nc.tensor.matmul(
                                        ps[:], wt[:, i, :], xslice(i, sc * 512, 512),
                                        start=(i == 0), stop=(i == 7),
                                    )
                                if half == 0:
                                    nc.vector.tensor_copy(out=dst[:, sl], in_=ps[:])
                                else:
                                    nc.vector.tensor_add(
                                        out=dst[:, sl], in0=ps[:], in1=dst[:, sl]
                                    )
                                    rope(dst, sl)
                        if (half == 0 and h >= 2) or half == 1:
                            emit_v({0: 8, 1: 4}[half])
                        if half == 0 and h == G_HEADS - 1:
                            # rope tables: needed from half 1 on
                            nc.scalar.dma_start(ce_t[:], ce_d)
                            nc.scalar.dma_start(s2_t[:], s2_d)

            # ---------------- phase 2: attention ----------------
            o_pool = tc.alloc_tile_pool(name="oT", bufs=1)
            o_t = [o_pool.tile([128, SEQ], F32R, tag=f"o{h}", name=f"o{h}") for h in range(G_HEADS)]
            wo_pool = tc.alloc_tile_pool(name="wo", bufs=1)
            wo_t = []
            for mc in range(G_HEADS):
                w = wo_pool.tile([128, SEQ], F32R, tag=f"wo{mc}", name=f"wo{mc}")
                nc.sync.dma_start(w[:], wo_d[mc])
                wo_t.append(w)
            with (
                tc.tile_pool(name="est", bufs=5) as est_pool,
                tc.tile_pool(name="nrm", bufs=3) as nrm_pool,
                tc.tile_pool(name="ps_st", bufs=2, space="PSUM") as ps_st,
                tc.tile_pool(name="ps_av", bufs=2, space="PSUM") as ps_av,
                tc.tile_pool(name="ps_dn", bufs=2, space="PSUM") as ps_dn,
            ):
                if with_mask:
                    mask_pool = tc.alloc_tile_pool(name="mask", bufs=2)

                for ic in range(SC512):
                    isl = bass.ts(ic, 512)
                    for hp in range(G_HEADS // 2):
                        heads = (2 * hp, 2 * hp + 1)
                        acc = {}
                        den = {}
                        e_of = {}
                        m_of = {}
                        for h in heads:
                            acc[h] = ps_av.tile([128, 512], F32, tag="acc", name="acc")
                            den[h] = ps_dn.tile([1, 512], F32, tag="den", name="den")

                        def emit_st(h, jc2):
                            ja, jb = 2 * jc2, 2 * jc2 + 1
                            st = ps_st.tile([128, 1024], F32, tag="st", name="st")
                            nc.tensor.matmul(
                                st[:, 0:512],
                                k_t[h][:, bass.ts(ja, 128)], q_t[h][:, isl],
                                start=True, stop=True,
                            )
                            nc.tensor.matmul(
                                st[:, 512:1024],
                                k_t[h][:, bass.ts(jb, 128)], q_t[h][:, isl],
                                start=True, stop=True,
                            )
                            e = est_pool.tile([128, 1024], F32R, tag="e", name="e")
                            if with_mask:
                                if jc2 not in m_of:
                                    mtl = mask_pool.tile(
                                        [128, 1024], F32, tag="m", name="mtl"
                                    )
                                    nc.sync.dma_start(mtl[:, 0:512], mt_d[ja, :, isl])
                                    nc.sync.dma_start(mtl[:, 512:1024], mt_d[jb, :, isl])
                                    m_of[jc2] = mtl
                                nc.vector.tensor_add(
                                    out=e[:], in0=st[:], in1=m_of[jc2][:]
                                )
                                nc.scalar.activation(
                                    out=e[:], in_=e[:],
                                    func=mybir.ActivationFunctionType.Exp,
                                )
                            else:
                                nc.scalar.activation(
                                    out=e[:], in_=st[:],
                                    func=mybir.ActivationFunctionType.Exp,
                                )
                            # pre-sum the two j-chunk halves on DVE so the
                            # denominator needs one PE matmul, not two
                            eh = est_pool.tile([128, 512], F32R, tag="eh", name="eh", bufs=3)
                            nc.vector.tensor_add(
                                out=eh[:], in0=e[:, 0:512], in1=e[:, 512:1024]
                            )
                            e_of[h] = (e, eh)

                        def emit_denav(h, jc2):
                            ja, jb = 2 * jc2, 2 * jc2 + 1
                            e, eh = e_of[h]
                            last = jc2 == SC128 // 2 - 1
                            nc.tensor.matmul(
                                den[h][:], ones_t[:], eh[:],
                                start=(jc2 == 0), stop=last,
                            )
                            nc.tensor.matmul(
                                acc[h][:], v_t[ja][:, bass.ts(h, 128)], e[:, 0:512],
                                start=(jc2 == 0), stop=False,
                            )
                            nc.tensor.matmul(
                                acc[h][:], v_t[jb][:, bass.ts(h, 128)], e[:, 512:1024],
                                start=False, stop=last,
                            )


                        # software pipeline: heads offset by a half step so PE
                        # always has independent matmuls while ACT runs exp
                        h0, h1 = heads
                        NJ2 = SC128 // 2
                        emit_st(h0, 0)
                        for jc2 in range(NJ2):
                            if jc2 > 0:
                                emit_st(h0, jc2)
                                emit_denav(h1, jc2 - 1)
                            emit_st(h1, jc2)
                            emit_denav(h0, jc2)
                        emit_denav(h1, NJ2 - 1)

                        for h in heads:
                            rec = nrm_pool.tile([1, 512], F32, tag="rec", name="rec")
                            nc.vector.reciprocal_approx_fast(out=rec[:], in_=den[h][:])
                            bc = nrm_pool.tile([128, 512], F32, tag="bc", name="bc")
                            nc.gpsimd.partition_broadcast(bc[:], rec[:])
                            nc.vector.tensor_mul(
                                out=o_t[h][:, isl], in0=acc[h][:], in1=bc[:]
                            )
                if with_mask:
                    mask_pool.release()

            # ---------------- phase 3: output projection ----------------
            with (
                tc.tile_pool(name="fin", bufs=10) as fin_pool,
                tc.tile_pool(name="ps3", bufs=8, space="PSUM") as ps3,
            ):
                for s in range(SC128):
                    ssl = bass.ts(s, 128)
                    for nck in range(SC512):
                        nsl = bass.ts(nck, 512)
                        ps = ps3.tile([128, 512], F32, tag="ps3", name="ps3")
                        for mc in range(G_HEADS):
                            nc.tensor.matmul(
                                ps[:], o_t[mc][:, ssl], wo_t[mc][:, nsl],
                                start=(mc == 0), stop=(mc == G_HEADS - 1),
                            )
                        f = fin_pool.tile([128, 512], F32, tag="f", name="f")
                        nc.vector.tensor_copy(out=f[:], in_=ps[:])
                        nc.sync.dma_start(out_d[s, :, nsl], f[:])
            wo_pool.release()
            o_pool.release()

    nc.compile()
    return nc


_CACHE = {}


def _get_nc(with_mask: bool):
    if with_mask not in _CACHE:
        _CACHE[with_mask] = build(with_mask)
    return _CACHE[with_mask]


def kernel(in_token, freqs_cos, freqs_sin, mask, wq, wk, wv, wo):
    return _run(in_token, freqs_cos, freqs_sin, mask, wq, wk, wv, wo)


def run_traced(in_token, freqs_cos, freqs_sin, mask, wq, wk, wv, wo):
    """Test-only: run with NTFF tracing, return (output, BassKernelResults)."""
    return _run(in_token, freqs_cos, freqs_sin, mask, wq, wk, wv, wo, trace=True)


def _run(in_token, freqs_cos, freqs_sin, mask, wq, wk, wv, wo, trace=False):
    in_token = np.ascontiguousarray(np.asarray(in_token, dtype=np.float32))
    freqs_cos = np.asarray(freqs_cos, dtype=np.float32)
    freqs_sin = np.asarray(freqs_sin, dtype=np.float32)
    mask = np.asarray(mask, dtype=np.float32)
    wq = np.asarray(wq, dtype=np.float32)
    wk = np.asarray(wk, dtype=np.float32)
    wv = np.asarray(wv, dtype=np.float32)
    wo = np.asarray(wo, dtype=np.float32)

    with_mask = bool(np.any(mask))
    nc = _get_nc(with_mask)

    # rope tables in (head_dim, seq) pair-expanded layout, signs/swap baked in
    ce = np.repeat(freqs_cos.T, 2, axis=0).astype(np.float32)  # (128, S)
    s2 = np.empty((HEAD_DIM, SEQ), np.float32)
    s2[0::2] = freqs_sin.T   # even rows: +sin (lands on odd out after swap)
    s2[1::2] = -freqs_sin.T  # odd rows: -sin (lands on even out after swap)
    ones = np.ones((128, 1), np.float32)
    if with_mask:
        mt = np.ascontiguousarray(mask.T).reshape(SC128, 128, SEQ)

    in_maps = []
    xts = [
        np.ascontiguousarray(in_token[b].T).reshape(DC, 128, SEQ)
        for b in range(BATCH)
    ]
    for b in range(BATCH):
        for g in range(G_HEADS):
            rows = slice(g * GM, (g + 1) * GM)
            wqt = np.ascontiguousarray(
                (wq[rows] * INV_SQRT_HD).T.reshape(
                    DC, 128, G_HEADS, 128
                ).transpose(0, 2, 1, 3)
            )
            wkt = np.ascontiguousarray(
                wk[rows].T.reshape(DC, 128, G_HEADS, 128).transpose(0, 2, 1, 3)
            )
            wvt = np.ascontiguousarray(wv[rows].T).reshape(DC, 128, GM)
            wot = np.ascontiguousarray(wo[:, rows].T).reshape(G_HEADS, 128, SEQ)
            m = {
                "xt": xts[b], "wq": wqt, "wk": wkt, "wv": wvt, "wo": wot,
                "ce": ce, "s2": s2, "ones": ones,
            }
            if with_mask:
                m["mt"] = mt
            in_maps.append(m)

    res = run_bass_kernel_spmd(nc, in_maps, core_ids=list(range(8)), trace=trace)

    out = np.zeros((BATCH, SEQ, DIM), np.float32)
    for b in range(BATCH):
        acc = None
        for g in range(G_HEADS):
            p = res.results[b * G_HEADS + g]["out"].reshape(SEQ, DIM)
            acc = p if acc is None else acc + p
        out[b] = acc
    if trace:
        return out, res
    return out



# revision 12
# speedup vs baseline: 1.0621x; 1.0621x over previous
"""Multi-head attention (B=2, S=2048, D=2048, H=16) on 8 Trainium2 cores.

Sharding: core = batch (2) x head-group (4 heads each). Tensor-parallel on
wq/wk/wv rows + wo columns; per-core partial outputs summed on host.

Device kernel (per core), all matmuls bf16 (fp32 PSUM accumulate):
  part 1: qT/kT (head_dim, seq) + v (seq, head_dim) projections in two
          contraction passes (dc 0..3, dc 4..15) so the PE starts before the
          x DMA completes; RoPE fused after the second pass (muls on gpsimd,
          shuffle+add on DVE).
  part 2: per (q-block, head) unit u: scores k-chunk matmuls -> exp (ACT) ->
          attn@V; softmax denominator via [128,1]-output matmuls (lhsT = e
          chunk, rhs = ones), normalized through tiny DMA transposes +
          partition broadcasts; the output-projection chains of the previous
          q-block are interleaved into the stream so the PE stays fed while
          ACT runs exp.
"""

import sys

for _p in ("/opt/trn_rl_repo",):
    if _p not in sys.path:
        sys.path.insert(0, _p)

import numpy as np
import ml_dtypes

import concourse.bass as bass
import concourse.tile as tile
from concourse import bacc, mybir
from concourse.bass_utils import run_bass_kernel_spmd

F32 = mybir.dt.float32
BF16 = mybir.dt.bfloat16
NPBF = ml_dtypes.bfloat16

DIM = 2048
N_HEADS = 16
HEAD_DIM = 128
BATCH = 2
SEQ = 2048
G_HEADS = 4          # heads per core
GM = G_HEADS * HEAD_DIM  # 512 output cols per core
DC = DIM // 128      # 16 contraction chunks
DCA = 4              # pass-A contraction chunks (dc 0..3)
DCB = DC - DCA       # pass-B chunks (dc 4..15)
SC512 = SEQ // 512   # 4
SC128 = SEQ // 128   # 16
NU = SC512 * G_HEADS  # 16 (q-block, head) units in part 2
INV_SQRT_HD = float(1.0 / np.sqrt(HEAD_DIM))

# even<->odd partition swap within each 32-partition group (rope pairs)
_SWAP_MASK = [i ^ 1 for i in range(32)]


def build(with_mask: bool, dbg: bool = False):
    nc = bacc.Bacc("TRN2", target_bir_lowering=False, debug=False)

    xt_d = nc.dram_tensor("xt", [DC, 128, SEQ], BF16, kind="ExternalInput").ap()
    wq_d = nc.dram_tensor("wq", [G_HEADS, 128, DC, 128], BF16, kind="ExternalInput").ap()
    wk_d = nc.dram_tensor("wk", [G_HEADS, 128, DC, 128], BF16, kind="ExternalInput").ap()
    wv_d = nc.dram_tensor("wv", [128, DC, GM], BF16, kind="ExternalInput").ap()
    wo_d = nc.dram_tensor("wo", [G_HEADS, 128, SEQ], BF16, kind="ExternalInput").ap()
    ce_d = nc.dram_tensor("ce", [128, SEQ], BF16, kind="ExternalInput").ap()
    s2_d = nc.dram_tensor("s2", [128, SEQ], BF16, kind="ExternalInput").ap()
    ones_d = nc.dram_tensor("ones", [128, 1], BF16, kind="ExternalInput").ap()
    mt_d = None
    if with_mask:
        mt_d = nc.dram_tensor("mt", [SC128, 128, SEQ], F32, kind="ExternalInput").ap()
    out_d = nc.dram_tensor("out", [SC128, 128, SEQ], F32, kind="ExternalOutput").ap()
    dbg_d = None
    if dbg:
        dbg_d = nc.dram_tensor("dbg", [4, 128, SEQ], BF16, kind="ExternalOutput").ap()

    with tile.TileContext(nc) as tc:
        persist = tc.alloc_tile_pool(name="persist", bufs=1)
        q_t = [persist.tile([128, SEQ], BF16, tag=f"q{h}", name=f"q{h}") for h in range(G_HEADS)]
        k_t = [persist.tile([128, SEQ], BF16, tag=f"k{h}", name=f"k{h}") for h in range(G_HEADS)]
        v_t = [persist.tile([128, GM], BF16, tag=f"v{s}", name=f"v{s}") for s in range(SC128)]
        o_t = [persist.tile([128, SEQ], BF16, tag=f"o{h}", name=f"o{h}") for h in range(G_HEADS)]
        wo_t = [persist.tile([128, SEQ], BF16, tag=f"wo{m}", name=f"wo{m}") for m in range(G_HEADS)]
        ce_t = persist.tile([128, SEQ], BF16, tag="ce", name="ce_t")
        s2_t = persist.tile([128, SEQ], BF16, tag="s2", name="s2_t")
        ones_t = persist.tile([128, 1], BF16, tag="ones", name="ones_t")
        warm_t = persist.tile([128, 1], F32, tag="warm", name="warm_t")

        xa_pool = tc.alloc_tile_pool(name="xa", bufs=1)
        xb_pool = tc.alloc_tile_pool(name="xb", bufs=1, side="right")
        wqk_pool = tc.alloc_tile_pool(name="wqk", bufs=2, side="right")
        rope_pool = tc.alloc_tile_pool(name="rope", bufs=2, side="right")
        ps1 = tc.alloc_tile_pool(name="ps1", bufs=4, space="PSUM")
        ps_st = tc.alloc_tile_pool(name="ps_st", bufs=2, space="PSUM", side="right")

        # ---------------- part 1, pass A: dc 0..3 ----------------
        xta = []
        with tc.high_priority():
            wt_first = wqk_pool.tile([128, DCA, 128], BF16, tag="wA", name="wtA")
            nc.sync.dma_start(wt_first[:], wq_d[0][:, 0:DCA, :])
            for dc in range(DCA):
                xa = xa_pool.tile([128, SEQ], BF16, tag=f"xa{dc}", name=f"xa{dc}")
                nc.sync.dma_start(xa[:], xt_d[dc])
                xta.append(xa)
        nc.sync.dma_start(ones_t[:], ones_d)
        nc.scalar.activation(
            out=warm_t[:], in_=ones_t[:], func=mybir.ActivationFunctionType.Exp
        )
        wva = xa_pool.tile([128, DCA, GM], BF16, tag="wvA", name="wvA")
        nc.sync.dma_start(wva[:], wv_d[:, 0:DCA, :])
        nc.sync.dma_start(ce_t[:], ce_d)
        nc.sync.dma_start(s2_t[:], s2_d)

        def qk_chain_a(h, wd, dst, wt=None):
            if wt is None:
                wt = wqk_pool.tile([128, DCA, 128], BF16, tag="wA", name="wtA")
                nc.sync.dma_start(wt[:], wd[h][:, 0:DCA, :])
            for sc in range(SC512):
                sl = bass.ts(sc, 512)
                ps = ps1.tile([128, 512], F32, tag="ps", name="ps")
                for i in range(DCA):
                    nc.tensor.matmul(
                        ps[:], wt[:, i, :], xta[i][:, sl],
                        start=(i == 0), stop=(i == DCA - 1),
                    )
                nc.vector.tensor_copy(out=dst[:, sl], in_=ps[:])

        def v_chain_a(s):
            ps = ps1.tile([128, GM], F32, tag="ps", name="ps")
            for i in range(DCA):
                nc.tensor.matmul(
                    ps[:], xta[i][:, bass.ts(s, 128)], wva[:, i, :],
                    start=(i == 0), stop=(i == DCA - 1),
                )
            nc.scalar.copy(out=v_t[s][:], in_=ps[:])

        xtb = []
        wvb_box = []

        def dma_xb(rng, with_wv=False):
            for dc in rng:
                xb = xb_pool.tile([128, SEQ], BF16, tag=f"xb{dc}", name=f"xb{dc}")
                nc.sync.dma_start(xb[:], xt_d[dc])
                xtb.append(xb)
            if with_wv:
                wvb = xb_pool.tile([128, DCB, GM], BF16, tag="wvB", name="wvB")
                nc.sync.dma_start(wvb[:], wv_d[:, DCA:DC, :])
                wvb_box.append(wvb)

        for h in range(G_HEADS):
            qk_chain_a(h, wq_d, q_t[h], wt=wt_first if h == 0 else None)
            qk_chain_a(h, wk_d, k_t[h])
            if h == 1:
                dma_xb(range(DCA, 10))
                for s in range(0, 8):
                    v_chain_a(s)
            if h == 3:
                dma_xb(range(10, DC), with_wv=True)
                for s in range(8, SC128):
                    v_chain_a(s)

        xa_pool.release()

        # pools that live from the end of part 1 through part 2
        est_pool = tc.alloc_tile_pool(name="est", bufs=14)
        nrm_pool = tc.alloc_tile_pool(name="nrm", bufs=2)
        fout_pool = tc.alloc_tile_pool(name="fout", bufs=2)
        if with_mask:
            mask_pool = tc.alloc_tile_pool(name="mask", bufs=4)

        e_of = {}

        def emit_st(u, j):
            ic, h = divmod(u, 4)
            isl = bass.ts(ic, 512)
            st = ps_st.tile([128, 512], F32, tag="st", name="st")
            nc.tensor.matmul(
                st[:], k_t[h][:, bass.ts(j, 128)], q_t[h][:, isl],
                start=True, stop=True,
            )
            if with_mask:
                mtl = mask_pool.tile([128, 512], F32, tag="m", name="mtl")
                nc.sync.dma_start(mtl[:], mt_d[j][:, isl])
                nc.vector.tensor_add(out=st[:], in0=st[:], in1=mtl[:])
            e = est_pool.tile([128, 512], BF16, tag="e", name="e")
            nc.scalar.activation(
                out=e[:], in_=st[:], func=mybir.ActivationFunctionType.Exp
            )
            e_of[(u, j)] = e

        # ---------------- part 1, pass B: dc 4..15 ----------------
        def rope(dst, sl):
            t1 = rope_pool.tile([128, 512], BF16, tag="t1", name="t1")
            nc.gpsimd.tensor_mul(out=t1[:], in0=dst[:, sl], in1=s2_t[:, sl])
            t3 = rope_pool.tile([128, 512], BF16, tag="t3", name="t3")
            nc.gpsimd.tensor_mul(out=t3[:], in0=dst[:, sl], in1=ce_t[:, sl])
            t2 = rope_pool.tile([128, 512], BF16, tag="t2", name="t2")
            nc.vector.stream_shuffle(t2[:], t1[:], _SWAP_MASK)
            nc.vector.tensor_add(out=dst[:, sl], in0=t2[:], in1=t3[:])

        def qk_chain_b(h, wd, dst):
            wt = wqk_pool.tile([128, DCB, 128], BF16, tag="wB", name="wtB")
            nc.sync.dma_start(wt[:], wd[h][:, DCA:DC, :])
            for sc in range(SC512):
                sl = bass.ts(sc, 512)
                ps = ps1.tile([128, 512], F32, tag="ps", name="ps")
                for i in range(DCB):
                    nc.tensor.matmul(
                        ps[:], wt[:, i, :], xtb[i][:, sl],
                        start=(i == 0), stop=(i == DCB - 1),
                    )
                nc.vector.tensor_add(out=dst[:, sl], in0=ps[:], in1=dst[:, sl])
                rope(dst, sl)

        def v_chain_b(s):
            wvb = wvb_box[0]
            ps = ps1.tile([128, GM], F32, tag="ps", name="ps")
            for i in range(DCB):
                nc.tensor.matmul(
                    ps[:], xtb[i][:, bass.ts(s, 128)], wvb[:, i, :],
                    start=(i == 0), stop=(i == DCB - 1),
                )
            nc.vector.tensor_add(out=v_t[s][:], in0=ps[:], in1=v_t[s][:])

        for h in range(G_HEADS):
            qk_chain_b(h, wq_d, q_t[h])
            qk_chain_b(h, wk_d, k_t[h])
            if h == 1:
                for s in range(0, 8):
                    v_chain_b(s)
            if h == 2:
                for m in range(G_HEADS):
                    nc.sync.dma_start(wo_t[m][:], wo_d[m])
            if h == 3:
                # interleave the first 8 scores of unit 0 into the v tail so
                # ACT gets a head start on exp before part 2 begins
                for s in range(8, SC128):
                    v_chain_b(s)
                    if s % 2 == 1:
                        j0 = (s - 8) // 2 * 2
                        emit_st(0, j0)
                        emit_st(0, j0 + 1)

        ps1.release()
        rope_pool.release()
        wqk_pool.release()
        xb_pool.release()

        # ---------------- part 2: attention + output projection ----------------
        ps_ap = tc.alloc_tile_pool(name="ps_ap", bufs=2, space="PSUM")
        ps_dn = tc.alloc_tile_pool(name="ps_dn", bufs=2, space="PSUM")

        fout_of = {}

        def p3_chain(pu, nck):
            pic, s_loc = divmod(pu, 4)
            s_glob = pic * 4 + s_loc
            pp = ps_ap.tile([128, 512], F32, tag="p3", name="pp")
            ssl = bass.ts(s_glob, 128)
            nsl = bass.ts(nck, 512)
            for m in range(G_HEADS):
                nc.tensor.matmul(
                    pp[:], o_t[m][:, ssl], wo_t[m][:, nsl],
                    start=(m == 0), stop=(m == G_HEADS - 1),
                )
            if nck == 0:
                fout_of[pu] = fout_pool.tile([128, SEQ], F32, tag="fout", name="fout")
            f = fout_of[pu]
            nc.vector.tensor_copy(out=f[:, nsl], in_=pp[:])
            if nck == SC512 - 1:
                nc.sync.dma_start(out_d[s_glob], f[:])
                del fout_of[pu]

        for u in range(NU):
            ic, h = divmod(u, 4)
            isl = bass.ts(ic, 512)
            hsl = bass.ts(h, 128)
            acc = ps_ap.tile([128, 512], F32, tag="acc", name="acc")
            den = ps_dn.tile([128, 4], F32, tag="den", name="den")
            for j in range(SC128):
                if j < 8:
                    emit_st(u, j + 8)
                elif u + 1 < NU:
                    emit_st(u + 1, j - 8)
                e = e_of.pop((u, j))
                nc.tensor.matmul(
                    acc[:], v_t[j][:, hsl], e[:],
                    start=(j == 0), stop=(j == SC128 - 1),
                )
                # one PSUM accumulation group for the whole [128,4] tile: the
                # 2KB zero-region auto-starts each byte range on first touch
                for qs in range(4):
                    nc.tensor.matmul(
                        den[:, qs : qs + 1], e[:, bass.ts(qs, 128)], ones_t[:],
                        start=(j == 0 and qs == 0),
                        stop=(j == SC128 - 1 and qs == 3),
                    )
                if j % 4 == 3 and u >= 4:
                    p3_chain(u - 4, j // 4)
            # normalize: rec = 1/den, transposed into a row, broadcast, mul
            rsb = nrm_pool.tile([128, 4], F32, tag="rsb", name="rsb")
            nc.vector.reciprocal_approx_fast(out=rsb[:], in_=den[:])
            bc = nrm_pool.tile([128, 512], F32, tag="bc", name="bc")
            for qs in range(4):
                rr = nrm_pool.tile([1, 128], F32, tag=f"rr{qs}", name="rr")
                nc.sync.dma_start(rr[:], rsb[:, qs : qs + 1])
                nc.gpsimd.partition_broadcast(bc[:, bass.ts(qs, 128)], rr[:])
            nc.vector.tensor_mul(out=o_t[h][:, isl], in0=acc[:], in1=bc[:])

        for pu in range(NU - 4, NU):
            for nck in range(SC512):
                p3_chain(pu, nck)

        if dbg:
            nc.sync.dma_start(dbg_d[0], q_t[0][:])
            nc.sync.dma_start(dbg_d[1], k_t[0][:])
            nc.sync.dma_start(dbg_d[2], o_t[0][:])
            nc.sync.dma_start(dbg_d[3][:, 0:GM], v_t[0][:])

        ps_dn.release()
        ps_ap.release()
        ps_st.release()
        if with_mask:
            mask_pool.release()
        fout_pool.release()
        nrm_pool.release()
        est_pool.release()
        persist.release()

    nc.compile()
    return nc


_CACHE = {}


def _get_nc(with_mask: bool):
    if with_mask not in _CACHE:
        _CACHE[with_mask] = build(with_mask)
    return _CACHE[with_mask]


def kernel(in_token, freqs_cos, freqs_sin, mask, wq, wk, wv, wo):
    return _run(in_token, freqs_cos, freqs_sin, mask, wq, wk, wv, wo)


def run_traced(in_token, freqs_cos, freqs_sin, mask, wq, wk, wv, wo):
    """Test-only: run with NTFF tracing, return (output, BassKernelResults)."""
    return _run(in_token, freqs_cos, freqs_sin, mask, wq, wk, wv, wo, trace=True)


def _run(in_token, freqs_cos, freqs_sin, mask, wq, wk, wv, wo, trace=False):
    in_token = np.asarray(in_token, dtype=np.float32)
    freqs_cos = np.asarray(freqs_cos, dtype=np.float32)
    freqs_sin = np.asarray(freqs_sin, dtype=np.float32)
    mask = np.asarray(mask, dtype=np.float32)
    wq = np.asarray(wq, dtype=np.float32)
    wk = np.asarray(wk, dtype=np.float32)
    wv = np.asarray(wv, dtype=np.float32)
    wo = np.asarray(wo, dtype=np.float32)

    with_mask = bool(np.any(mask))
    nc = _get_nc(with_mask)

    # rope tables in (head_dim, seq) pair-expanded layout, signs/swap baked in
    ce = np.repeat(freqs_cos.T, 2, axis=0).astype(NPBF)  # (128, S)
    s2 = np.empty((HEAD_DIM, SEQ), np.float32)
    s2[0::2] = freqs_sin.T   # even rows: +sin (lands on odd out after swap)
    s2[1::2] = -freqs_sin.T  # odd rows: -sin (lands on even out after swap)
    s2 = s2.astype(NPBF)
    ones = np.ones((128, 1), NPBF)
    if with_mask:
        mt = np.ascontiguousarray(mask.T).reshape(SC128, 128, SEQ)

    xts = [
        np.ascontiguousarray(in_token[b].T).astype(NPBF).reshape(DC, 128, SEQ)
        for b in range(BATCH)
    ]
    # per-head-group weight layouts (shared across the two batch cores)
    gmaps = []
    for g in range(G_HEADS):
        rows = slice(g * GM, (g + 1) * GM)
        # wt[h, p, dc, m] = w[g*512 + h*128 + m, dc*128 + p]
        wqt = np.ascontiguousarray(
            (wq[rows] * INV_SQRT_HD).reshape(G_HEADS, 128, DC, 128).transpose(0, 3, 2, 1)
        ).astype(NPBF)
        wkt = np.ascontiguousarray(
            wk[rows].reshape(G_HEADS, 128, DC, 128).transpose(0, 3, 2, 1)
        ).astype(NPBF)
        # wvt[p, dc, n] = wv[g*512 + n, dc*128 + p]
        wvt = np.ascontiguousarray(
            wv[rows].reshape(GM, DC, 128).transpose(2, 1, 0)
        ).astype(NPBF)
        # wot[mc, hd, n] = wo[n, g*512 + mc*128 + hd]
        wot = np.ascontiguousarray(wo[:, rows].T).astype(NPBF).reshape(G_HEADS, 128, SEQ)
        gmaps.append({"wq": wqt, "wk": wkt, "wv": wvt, "wo": wot})

    in_maps = []
    for b in range(BATCH):
        for g in range(G_HEADS):
            m = {
                "xt": xts[b], "ce": ce, "s2": s2, "ones": ones, **gmaps[g],
            }
            if with_mask:
                m["mt"] = mt
            in_maps.append(m)

    res = run_bass_kernel_spmd(nc, in_maps, core_ids=list(range(8)), trace=trace)

    out = np.zeros((BATCH, SEQ, DIM), np.float32)
    for b in range(BATCH):
        acc = None
        for g in range(G_HEADS):
            p = res.results[b * G_HEADS + g]["out"].reshape(SEQ, DIM)
            acc = p if acc is None else acc + p
        out[b] = acc
    if trace:
        return out, res
    return out


# revision 17
# speedup vs baseline: 1.1106x; 1.0457x over previous
"""Multi-head attention (B=2, S=2048, D=2048, H=16) on 8 Trainium2 cores.

Sharding: core = batch (2) x head-group (4 heads each). Tensor-parallel on
wq/wk/wv rows + wo columns; per-core partial outputs summed on host.

Device kernel (per core), all matmuls bf16 (fp32 PSUM accumulate):
  part 1: qT/kT (head_dim, seq) + v (seq, head_dim) projections in two
          contraction passes (dc 0..3, dc 4..15) so the PE starts before the
          x DMA completes; RoPE fused after the second pass (muls on gpsimd,
          shuffle+adds on DVE), software-pipelined one chain behind the PE so
          the DVE->gpsimd->DVE dependency line never paces the PE.
  part 2: per (q-block, head) unit u: scores k-chunk-pair matmuls into
          [128,1024] PSUM -> exp (ACT) -> attn@V; softmax denominator via
          [128,1]-output matmuls (lhsT = e chunk, rhs = ones) sharing one
          PSUM accumulation group, normalized through tiny DMA transposes +
          partition broadcasts; the output-projection chains of q-block-4-ago
          are interleaved into the stream so the PE stays fed while ACT exps.
"""

import sys

for _p in ("/opt/trn_rl_repo",):
    if _p not in sys.path:
        sys.path.insert(0, _p)

import numpy as np
import ml_dtypes

import concourse.bass as bass
import concourse.tile as tile
from concourse import bacc, mybir
from concourse.bass_utils import run_bass_kernel_spmd

F32 = mybir.dt.float32
BF16 = mybir.dt.bfloat16
NPBF = ml_dtypes.bfloat16

DIM = 2048
N_HEADS = 16
HEAD_DIM = 128
BATCH = 2
SEQ = 2048
G_HEADS = 4          # heads per core
GM = G_HEADS * HEAD_DIM  # 512 output cols per core
DC = DIM // 128      # 16 contraction chunks
DCA = 4              # pass-A contraction chunks (dc 0..3)
DCB = DC - DCA       # pass-B chunks (dc 4..15)
SC512 = SEQ // 512   # 4
SC128 = SEQ // 128   # 16
NJ2 = SC128 // 2     # 8 k-chunk pairs
NU = SC512 * G_HEADS  # 16 (q-block, head) units in part 2
INV_SQRT_HD = float(1.0 / np.sqrt(HEAD_DIM))

# even<->odd partition swap within each 32-partition group (rope pairs)
_SWAP_MASK = [i ^ 1 for i in range(32)]


def build(with_mask: bool, dbg: bool = False):
    nc = bacc.Bacc("TRN2", target_bir_lowering=False, debug=False)

    xt_d = nc.dram_tensor("xt", [DC, 128, SEQ], BF16, kind="ExternalInput").ap()
    wq_d = nc.dram_tensor("wq", [G_HEADS, 128, DC, 128], BF16, kind="ExternalInput").ap()
    wk_d = nc.dram_tensor("wk", [G_HEADS, 128, DC, 128], BF16, kind="ExternalInput").ap()
    wv_d = nc.dram_tensor("wv", [128, DC, GM], BF16, kind="ExternalInput").ap()
    wo_d = nc.dram_tensor("wo", [G_HEADS, 128, SEQ], BF16, kind="ExternalInput").ap()
    ce_d = nc.dram_tensor("ce", [128, SEQ], BF16, kind="ExternalInput").ap()
    s2_d = nc.dram_tensor("s2", [128, SEQ], BF16, kind="ExternalInput").ap()
    ones_d = nc.dram_tensor("ones", [128, 1], BF16, kind="ExternalInput").ap()
    mt_d = None
    if with_mask:
        mt_d = nc.dram_tensor("mt", [SC128, 128, SEQ], F32, kind="ExternalInput").ap()
    out_d = nc.dram_tensor("out", [SC128, 128, SEQ], F32, kind="ExternalOutput").ap()
    dbg_d = None
    if dbg:
        dbg_d = nc.dram_tensor("dbg", [4, 128, SEQ], BF16, kind="ExternalOutput").ap()

    with tile.TileContext(nc) as tc:
        persist = tc.alloc_tile_pool(name="persist", bufs=1)
        q_t = [persist.tile([128, SEQ], BF16, tag=f"q{h}", name=f"q{h}") for h in range(G_HEADS)]
        k_t = [persist.tile([128, SEQ], BF16, tag=f"k{h}", name=f"k{h}") for h in range(G_HEADS)]
        v_t = [persist.tile([128, GM], BF16, tag=f"v{s}", name=f"v{s}") for s in range(SC128)]
        o_t = [persist.tile([128, SEQ], BF16, tag=f"o{h}", name=f"o{h}") for h in range(G_HEADS)]
        wo_t = [persist.tile([128, SEQ], BF16, tag=f"wo{m}", name=f"wo{m}") for m in range(G_HEADS)]
        ce_t = persist.tile([128, SEQ], BF16, tag="ce", name="ce_t")
        s2_t = persist.tile([128, SEQ], BF16, tag="s2", name="s2_t")
        ones_t = persist.tile([128, 1], BF16, tag="ones", name="ones_t")
        warm_t = persist.tile([128, 1], F32, tag="warm", name="warm_t")

        xa_pool = tc.alloc_tile_pool(name="xa", bufs=1)
        xb_pool = tc.alloc_tile_pool(name="xb", bufs=1, side="right")
        wqk_pool = tc.alloc_tile_pool(name="wqk", bufs=2, side="right")
        rope_pool = tc.alloc_tile_pool(name="rope", bufs=2, side="right")
        ps1 = tc.alloc_tile_pool(name="ps1", bufs=4, space="PSUM")
        ps_st = tc.alloc_tile_pool(name="ps_st", bufs=2, space="PSUM", side="right")

        # ---------------- part 1, pass A: dc 0..3 ----------------
        xta = []
        with tc.high_priority():
            wt_first = wqk_pool.tile([128, DCA, 128], BF16, tag="wA", name="wtA")
            nc.sync.dma_start(wt_first[:], wq_d[0][:, 0:DCA, :])
            for dc in range(DCA):
                xa = xa_pool.tile([128, SEQ], BF16, tag=f"xa{dc}", name=f"xa{dc}")
                nc.sync.dma_start(xa[:], xt_d[dc])
                xta.append(xa)
        wt_k0 = wqk_pool.tile([128, DCA, 128], BF16, tag="wA", name="wtA")
        nc.sync.dma_start(wt_k0[:], wk_d[0][:, 0:DCA, :])
        nc.sync.dma_start(ones_t[:], ones_d)
        nc.scalar.activation(
            out=warm_t[:], in_=ones_t[:], func=mybir.ActivationFunctionType.Exp
        )
        wva = xa_pool.tile([128, DCA, GM], BF16, tag="wvA", name="wvA")
        nc.sync.dma_start(wva[:], wv_d[:, 0:DCA, :])

        def qk_chain_a(h, wd, dst, wt=None):
            if wt is None:
                wt = wqk_pool.tile([128, DCA, 128], BF16, tag="wA", name="wtA")
                nc.sync.dma_start(wt[:], wd[h][:, 0:DCA, :])
            for sc in range(SC512):
                sl = bass.ts(sc, 512)
                ps = ps1.tile([128, 512], F32, tag="ps", name="ps")
                for i in range(DCA):
                    nc.tensor.matmul(
                        ps[:], wt[:, i, :], xta[i][:, sl],
                        start=(i == 0), stop=(i == DCA - 1),
                    )
                nc.vector.tensor_copy(out=dst[:, sl], in_=ps[:])

        def v_chain_a(s):
            ps = ps1.tile([128, GM], F32, tag="ps", name="ps")
            for i in range(DCA):
                nc.tensor.matmul(
                    ps[:], xta[i][:, bass.ts(s, 128)], wva[:, i, :],
                    start=(i == 0), stop=(i == DCA - 1),
                )
            nc.scalar.copy(out=v_t[s][:], in_=ps[:])

        xtb = []
        wvb_box = []

        def dma_xb(rng, with_wv=False):
            for dc in rng:
                xb = xb_pool.tile([128, SEQ], BF16, tag=f"xb{dc}", name=f"xb{dc}")
                nc.sync.dma_start(xb[:], xt_d[dc])
                xtb.append(xb)
            if with_wv:
                wvb = xb_pool.tile([128, DCB, GM], BF16, tag="wvB", name="wvB")
                nc.sync.dma_start(wvb[:], wv_d[:, DCA:DC, :])
                wvb_box.append(wvb)

        for h in range(G_HEADS):
            qk_chain_a(h, wq_d, q_t[h], wt=wt_first if h == 0 else None)
            qk_chain_a(h, wk_d, k_t[h], wt=wt_k0 if h == 0 else None)
            if h == 1:
                dma_xb(range(DCA, 10))
                for s in range(0, 8):
                    v_chain_a(s)
            if h == 2:
                # rope tables: first needed at the start of pass B
                nc.sync.dma_start(ce_t[:], ce_d)
                nc.sync.dma_start(s2_t[:], s2_d)
            if h == 3:
                dma_xb(range(10, DC), with_wv=True)
                for s in range(8, SC128):
                    v_chain_a(s)

        xa_pool.release()

        # lives from the end of part 1 through part 2
        est_pool = tc.alloc_tile_pool(name="est", bufs=10)
        if with_mask:
            mask_pool = tc.alloc_tile_pool(name="mask", bufs=4)

        e_of = {}

        def emit_st(u, jc2):
            # scores for k-chunk pair (2*jc2, 2*jc2+1) of unit u, then exp
            ic, h = divmod(u, 4)
            isl = bass.ts(ic, 512)
            st = ps_st.tile([128, 1024], F32, tag="st", name="st")
            for half in range(2):
                j = 2 * jc2 + half
                nc.tensor.matmul(
                    st[:, bass.ts(half, 512)],
                    k_t[h][:, bass.ts(j, 128)], q_t[h][:, isl],
                    start=True, stop=True,
                )
            if with_mask:
                mtl = mask_pool.tile([128, 1024], F32, tag="m", name="mtl")
                for half in range(2):
                    j = 2 * jc2 + half
                    nc.sync.dma_start(mtl[:, bass.ts(half, 512)], mt_d[j][:, isl])
                nc.vector.tensor_add(out=st[:], in0=st[:], in1=mtl[:])
            e = est_pool.tile([128, 1024], BF16, tag="e", name="e")
            nc.scalar.activation(
                out=e[:], in_=st[:], func=mybir.ActivationFunctionType.Exp
            )
            e_of[(u, jc2)] = e

        # ---------------- part 1, pass B: dc 4..15 ----------------
        # rope is pipelined one chain behind: emit_tail flushes the shuffle+add
        # of the previous chain so the PE never waits on the DVE/gpsimd line
        rope_pend = []

        def rope_head(dst, sl):
            t1 = rope_pool.tile([128, 512], BF16, tag="t1", name="t1")
            nc.gpsimd.tensor_mul(out=t1[:], in0=dst[:, sl], in1=s2_t[:, sl])
            t3 = rope_pool.tile([128, 512], BF16, tag="t3", name="t3")
            nc.gpsimd.tensor_mul(out=t3[:], in0=dst[:, sl], in1=ce_t[:, sl])
            rope_pend.append((dst, sl, t1, t3))

        def rope_tail():
            while rope_pend:
                dst, sl, t1, t3 = rope_pend.pop(0)
                t2 = rope_pool.tile([128, 512], BF16, tag="t2", name="t2")
                nc.vector.stream_shuffle(t2[:], t1[:], _SWAP_MASK)
                nc.vector.tensor_add(out=dst[:, sl], in0=t2[:], in1=t3[:])

        def qk_chain_b(h, wd, dst):
            wt = wqk_pool.tile([128, DCB, 128], BF16, tag="wB", name="wtB")
            nc.sync.dma_start(wt[:], wd[h][:, DCA:DC, :])
            for sc in range(SC512):
                sl = bass.ts(sc, 512)
                ps = ps1.tile([128, 512], F32, tag="ps", name="ps")
                for i in range(DCB):
                    nc.tensor.matmul(
                        ps[:], wt[:, i, :], xtb[i][:, sl],
                        start=(i == 0), stop=(i == DCB - 1),
                    )
                nc.vector.tensor_add(out=dst[:, sl], in0=ps[:], in1=dst[:, sl])
                rope_head(dst, sl)
                rope_tail_deferred()

        # one-chain-deep deferral: flush all but the newest pending rope
        def rope_tail_deferred():
            while len(rope_pend) > 1:
                dst, sl, t1, t3 = rope_pend.pop(0)
                t2 = rope_pool.tile([128, 512], BF16, tag="t2", name="t2")
                nc.vector.stream_shuffle(t2[:], t1[:], _SWAP_MASK)
                nc.vector.tensor_add(out=dst[:, sl], in0=t2[:], in1=t3[:])

        def v_chain_b(s):
            wvb = wvb_box[0]
            ps = ps1.tile([128, GM], F32, tag="ps", name="ps")
            for i in range(DCB):
                nc.tensor.matmul(
                    ps[:], xtb[i][:, bass.ts(s, 128)], wvb[:, i, :],
                    start=(i == 0), stop=(i == DCB - 1),
                )
            nc.vector.tensor_add(out=v_t[s][:], in0=ps[:], in1=v_t[s][:])

        for h in range(G_HEADS):
            qk_chain_b(h, wq_d, q_t[h])
            qk_chain_b(h, wk_d, k_t[h])
            if h == 1:
                for s in range(0, 8):
                    v_chain_b(s)
            if h == 2:
                for m in range(G_HEADS):
                    nc.sync.dma_start(wo_t[m][:], wo_d[m])
            if h == 3:
                rope_tail()
                # interleave the first 4 score-pairs of unit 0 into the v tail
                # so ACT gets a head start on exp before part 2 begins
                for s in range(8, SC128):
                    v_chain_b(s)
                    if s % 2 == 1:
                        emit_st(0, (s - 9) // 2)
        rope_tail()

        ps1.release()
        rope_pool.release()
        wqk_pool.release()
        xb_pool.release()

        # ---------------- part 2: attention + output projection ----------------
        ps_ap = tc.alloc_tile_pool(name="ps_ap", bufs=2, space="PSUM")
        ps_dn = tc.alloc_tile_pool(name="ps_dn", bufs=1, space="PSUM")
        nrm_pool = tc.alloc_tile_pool(name="nrm", bufs=2)
        fout_pool = tc.alloc_tile_pool(name="fout", bufs=2)

        fout_of = {}

        def p3_chain(pu, nck, small_dma=False):
            pic, s_loc = divmod(pu, 4)
            s_glob = pic * 4 + s_loc
            pp = ps_ap.tile([128, 512], F32, tag="p3", name="pp", bufs=1)
            ssl = bass.ts(s_glob, 128)
            nsl = bass.ts(nck, 512)
            for m in range(G_HEADS):
                nc.tensor.matmul(
                    pp[:], o_t[m][:, ssl], wo_t[m][:, nsl],
                    start=(m == 0), stop=(m == G_HEADS - 1),
                )
            if small_dma:
                f = fout_pool.tile([128, 512], F32, tag="fs", name="fs", bufs=2)
                nc.vector.tensor_copy(out=f[:], in_=pp[:])
                nc.sync.dma_start(out_d[s_glob][:, nsl], f[:])
                return
            if nck == 0:
                fout_of[pu] = fout_pool.tile([128, SEQ], F32, tag="fout", name="fout")
            f = fout_of[pu]
            nc.vector.tensor_copy(out=f[:, nsl], in_=pp[:])
            if nck == SC512 - 1:
                nc.sync.dma_start(out_d[s_glob], f[:])
                del fout_of[pu]

        def norm(u):
            ic, h = divmod(u, 4)
            isl = bass.ts(ic, 512)
            den, acc = den_acc[u]
            rsb = nrm_pool.tile([128, 4], F32, tag="rsb", name="rsb")
            nc.vector.reciprocal_approx_fast(out=rsb[:], in_=den[:])
            bc = nrm_pool.tile([128, 512], F32, tag="bc", name="bc")
            for qs in range(4):
                rr = nrm_pool.tile([1, 128], F32, tag=f"rr{qs}", name="rr")
                nc.sync.dma_start(rr[:], rsb[:, qs : qs + 1])
                nc.gpsimd.partition_broadcast(bc[:, bass.ts(qs, 128)], rr[:])
            nc.vector.tensor_mul(out=o_t[h][:, isl], in0=acc[:], in1=bc[:])

        den_acc = {}
        for u in range(NU):
            ic, h = divmod(u, 4)
            hsl = bass.ts(h, 128)
            acc = ps_ap.tile([128, 512], F32, tag="acc", name="acc")
            den = ps_dn.tile([128, 4], F32, tag="den", name="den")
            den_acc[u] = (den, acc)
            last_u = u == NU - 1
            for jc2 in range(NJ2):
                if jc2 < 4:
                    emit_st(u, jc2 + 4)
                elif not last_u:
                    emit_st(u + 1, jc2 - 4)
                e = e_of.pop((u, jc2))
                for half in range(2):
                    j = 2 * jc2 + half
                    esl = e[:, bass.ts(half, 512)]
                    nc.tensor.matmul(
                        acc[:], v_t[j][:, hsl], esl,
                        start=(j == 0), stop=(j == SC128 - 1),
                    )
                    # one PSUM accumulation group for the whole [128,4] tile:
                    # the 2KB zero-region auto-starts each byte on first touch
                    for qs in range(4):
                        nc.tensor.matmul(
                            den[:, qs : qs + 1],
                            e[:, half * 512 + qs * 128 : half * 512 + (qs + 1) * 128],
                            ones_t[:],
                            start=(j == 0 and qs == 0),
                            stop=(j == SC128 - 1 and qs == 3),
                        )
                if jc2 == NJ2 - 1:
                    norm(u)
                if jc2 % 2 == 1 and u >= 4:
                    p3_chain(u - 4, jc2 // 2)

        # tail: output projections for q-block 3 (units 12..15)
        for pu in range(NU - 4, NU):
            for nck in range(SC512):
                p3_chain(pu, nck, small_dma=True)

        if dbg:
            nc.sync.dma_start(dbg_d[0], q_t[0][:])
            nc.sync.dma_start(dbg_d[1], k_t[0][:])
            nc.sync.dma_start(dbg_d[2], o_t[0][:])
            nc.sync.dma_start(dbg_d[3][:, 0:GM], v_t[0][:])

        ps_dn.release()
        ps_ap.release()
        ps_st.release()
        fout_pool.release()
        nrm_pool.release()
        if with_mask:
            mask_pool.release()
        est_pool.release()
        persist.release()

    nc.compile()
    return nc


_CACHE = {}


def _get_nc(with_mask: bool):
    if with_mask not in _CACHE:
        _CACHE[with_mask] = build(with_mask)
    return _CACHE[with_mask]


def kernel(in_token, freqs_cos, freqs_sin, mask, wq, wk, wv, wo):
    return _run(in_token, freqs_cos, freqs_sin, mask, wq, wk, wv, wo)


def run_traced(in_token, freqs_cos, freqs_sin, mask, wq, wk, wv, wo):
    """Test-only: run with NTFF tracing, return (output, BassKernelResults)."""
    return _run(in_token, freqs_cos, freqs_sin, mask, wq, wk, wv, wo, trace=True)


def _run(in_token, freqs_cos, freqs_sin, mask, wq, wk, wv, wo, trace=False):
    in_token = np.asarray(in_token, dtype=np.float32)
    freqs_cos = np.asarray(freqs_cos, dtype=np.float32)
    freqs_sin = np.asarray(freqs_sin, dtype=np.float32)
    mask = np.asarray(mask, dtype=np.float32)
    wq = np.asarray(wq, dtype=np.float32)
    wk = np.asarray(wk, dtype=np.float32)
    wv = np.asarray(wv, dtype=np.float32)
    wo = np.asarray(wo, dtype=np.float32)

    with_mask = bool(np.any(mask))
    nc = _get_nc(with_mask)

    # rope tables in (head_dim, seq) pair-expanded layout, signs/swap baked in
    ce = np.repeat(freqs_cos.T, 2, axis=0).astype(NPBF)  # (128, S)
    s2 = np.empty((HEAD_DIM, SEQ), np.float32)
    s2[0::2] = freqs_sin.T   # even rows: +sin (lands on odd out after swap)
    s2[1::2] = -freqs_sin.T  # odd rows: -sin (lands on even out after swap)
    s2 = s2.astype(NPBF)
    ones = np.ones((128, 1), NPBF)
    if with_mask:
        mt = np.ascontiguousarray(mask.T).reshape(SC128, 128, SEQ)

    xts = [
        np.ascontiguousarray(in_token[b].T).astype(NPBF).reshape(DC, 128, SEQ)
        for b in range(BATCH)
    ]
    # per-head-group weight layouts (shared across the two batch cores)
    gmaps = []
    for g in range(G_HEADS):
        rows = slice(g * GM, (g + 1) * GM)
        # wt[h, p, dc, m] = w[g*512 + h*128 + m, dc*128 + p]
        wqt = np.ascontiguousarray(
            (wq[rows] * INV_SQRT_HD).reshape(G_HEADS, 128, DC, 128).transpose(0, 3, 2, 1)
        ).astype(NPBF)
        wkt = np.ascontiguousarray(
            wk[rows].reshape(G_HEADS, 128, DC, 128).transpose(0, 3, 2, 1)
        ).astype(NPBF)
        # wvt[p, dc, n] = wv[g*512 + n, dc*128 + p]
        wvt = np.ascontiguousarray(
            wv[rows].reshape(GM, DC, 128).transpose(2, 1, 0)
        ).astype(NPBF)
        # wot[mc, hd, n] = wo[n, g*512 + mc*128 + hd]
        wot = np.ascontiguousarray(wo[:, rows].T).astype(NPBF).reshape(G_HEADS, 128, SEQ)
        gmaps.append({"wq": wqt, "wk": wkt, "wv": wvt, "wo": wot})

    in_maps = []
    for b in range(BATCH):
        for g in range(G_HEADS):
            m = {
                "xt": xts[b], "ce": ce, "s2": s2, "ones": ones, **gmaps[g],
            }
            if with_mask:
                m["mt"] = mt
            in_maps.append(m)

    res = run_bass_kernel_spmd(nc, in_maps, core_ids=list(range(8)), trace=trace)

    out = np.zeros((BATCH, SEQ, DIM), np.float32)
    for b in range(BATCH):
        acc = None
        for g in range(G_HEADS):
            p = res.results[b * G_HEADS + g]["out"].reshape(SEQ, DIM)
            acc = p if acc is None else acc + p
        out[b] = acc
    if trace:
        return out, res
    return out


# revision 22
# speedup vs baseline: 1.1296x; 1.0171x over previous
"""Multi-head attention (B=2, S=2048, D=2048, H=16) on 8 Trainium2 cores.

Sharding: core = batch (2) x head-group (4 heads each). Tensor-parallel on
wq/wk/wv rows + wo columns; per-core partial outputs summed on host.

Device kernel (per core), all matmuls bf16 (fp32 PSUM accumulate):
  part 1: qT/kT (head_dim, seq) + v (seq, head_dim) projections in two
          contraction passes (dc 0..3, dc 4..15) so the PE starts before the
          x DMA completes; RoPE fused after the second pass (muls on gpsimd,
          shuffle+adds on DVE), software-pipelined one chain behind the PE so
          the DVE->gpsimd->DVE dependency line never paces the PE.
  part 2: per (q-block, head) unit u: scores k-chunk-pair matmuls into
          [128,1024] PSUM -> exp (ACT) -> attn@V; softmax denominator via
          [128,1]-output matmuls (lhsT = e chunk, rhs = ones) sharing one
          PSUM accumulation group, normalized through tiny DMA transposes +
          partition broadcasts; the output-projection chains of q-block-4-ago
          are interleaved into the stream so the PE stays fed while ACT exps.
"""

import sys

for _p in ("/opt/trn_rl_repo",):
    if _p not in sys.path:
        sys.path.insert(0, _p)

import numpy as np
import ml_dtypes

import concourse.bass as bass
import concourse.tile as tile
from concourse import bacc, mybir
from concourse.bass_utils import run_bass_kernel_spmd

F32 = mybir.dt.float32
BF16 = mybir.dt.bfloat16
NPBF = ml_dtypes.bfloat16

DIM = 2048
N_HEADS = 16
HEAD_DIM = 128
BATCH = 2
SEQ = 2048
G_HEADS = 4          # heads per core
GM = G_HEADS * HEAD_DIM  # 512 output cols per core
DC = DIM // 128      # 16 contraction chunks
DCA = 4              # pass-A contraction chunks (dc 0..3)
DCB = DC - DCA       # pass-B chunks (dc 4..15)
SC512 = SEQ // 512   # 4
SC128 = SEQ // 128   # 16
NJ2 = SC128 // 2     # 8 k-chunk pairs
NU = SC512 * G_HEADS  # 16 (q-block, head) units in part 2
INV_SQRT_HD = float(1.0 / np.sqrt(HEAD_DIM))

# even<->odd partition swap within each 32-partition group (rope pairs)
_SWAP_MASK = [i ^ 1 for i in range(32)]


def build(with_mask: bool, dbg: bool = False):
    nc = bacc.Bacc("TRN2", target_bir_lowering=False, debug=False)

    xt_d = nc.dram_tensor("xt", [DC, 128, SEQ], BF16, kind="ExternalInput").ap()
    wq_d = nc.dram_tensor("wq", [G_HEADS, 128, DC, 128], BF16, kind="ExternalInput").ap()
    wk_d = nc.dram_tensor("wk", [G_HEADS, 128, DC, 128], BF16, kind="ExternalInput").ap()
    wv_d = nc.dram_tensor("wv", [128, DC, GM], BF16, kind="ExternalInput").ap()
    wo_d = nc.dram_tensor("wo", [G_HEADS, 128, SEQ], BF16, kind="ExternalInput").ap()
    ce_d = nc.dram_tensor("ce", [128, SEQ], BF16, kind="ExternalInput").ap()
    s2_d = nc.dram_tensor("s2", [128, SEQ], BF16, kind="ExternalInput").ap()
    ones_d = nc.dram_tensor("ones", [128, 1], BF16, kind="ExternalInput").ap()
    mt_d = None
    if with_mask:
        mt_d = nc.dram_tensor("mt", [SC128, 128, SEQ], F32, kind="ExternalInput").ap()
    out_d = nc.dram_tensor("out", [SC128, 128, SEQ], F32, kind="ExternalOutput").ap()
    dbg_d = None
    if dbg:
        dbg_d = nc.dram_tensor("dbg", [4, 128, SEQ], BF16, kind="ExternalOutput").ap()

    with tile.TileContext(nc) as tc:
        persist = tc.alloc_tile_pool(name="persist", bufs=1)
        q_t = [persist.tile([128, SEQ], BF16, tag=f"q{h}", name=f"q{h}") for h in range(G_HEADS)]
        k_t = [persist.tile([128, SEQ], BF16, tag=f"k{h}", name=f"k{h}") for h in range(G_HEADS)]
        v_t = [persist.tile([128, GM], BF16, tag=f"v{s}", name=f"v{s}") for s in range(SC128)]
        o_t = [persist.tile([128, SEQ], BF16, tag=f"o{h}", name=f"o{h}") for h in range(G_HEADS)]
        wo_t = [persist.tile([128, SEQ], BF16, tag=f"wo{m}", name=f"wo{m}") for m in range(G_HEADS)]
        ce_t = persist.tile([128, SEQ], BF16, tag="ce", name="ce_t")
        s2_t = persist.tile([128, SEQ], BF16, tag="s2", name="s2_t")
        ones_t = persist.tile([128, 1], BF16, tag="ones", name="ones_t")
        warm_t = persist.tile([128, 1], F32, tag="warm", name="warm_t")

        xa_pool = tc.alloc_tile_pool(name="xa", bufs=1)
        xb_pool = tc.alloc_tile_pool(name="xb", bufs=1, side="right")
        wqk_pool = tc.alloc_tile_pool(name="wqk", bufs=2, side="right")
        rope_pool = tc.alloc_tile_pool(name="rope", bufs=3, side="right")
        ps1 = tc.alloc_tile_pool(name="ps1", bufs=4, space="PSUM")
        ps_st = tc.alloc_tile_pool(name="ps_st", bufs=2, space="PSUM", side="right")

        # ---------------- part 1, pass A: dc 0..3 ----------------
        # x tiles split into seq-halves so the first chains unblock after
        # half the startup DMA bytes
        xta0, xta1 = [], []
        with tc.high_priority():
            wt_first = wqk_pool.tile([128, DCA, 128], BF16, tag="wA", name="wtA")
            nc.sync.dma_start(wt_first[:], wq_d[0][:, 0:DCA, :])
            for dc in range(DCA):
                xa = xa_pool.tile([128, 1024], BF16, tag=f"xa{dc}a", name=f"xa{dc}a")
                nc.sync.dma_start(xa[:], xt_d[dc][:, 0:1024])
                xta0.append(xa)
        for dc in range(DCA):
            xa = xa_pool.tile([128, 1024], BF16, tag=f"xa{dc}b", name=f"xa{dc}b")
            nc.sync.dma_start(xa[:], xt_d[dc][:, 1024:2048])
            xta1.append(xa)

        def xa_slice(i, start, width):
            if start < 1024:
                return xta0[i][:, start : start + width]
            return xta1[i][:, start - 1024 : start - 1024 + width]
        wt_k0 = wqk_pool.tile([128, DCA, 128], BF16, tag="wA", name="wtA")
        nc.sync.dma_start(wt_k0[:], wk_d[0][:, 0:DCA, :])
        nc.sync.dma_start(ones_t[:], ones_d)
        nc.scalar.activation(
            out=warm_t[:], in_=ones_t[:], func=mybir.ActivationFunctionType.Exp
        )
        wva = xa_pool.tile([128, DCA, GM], BF16, tag="wvA", name="wvA")
        nc.sync.dma_start(wva[:], wv_d[:, 0:DCA, :])

        def qk_chain_a(h, wd, dst, wt=None):
            if wt is None:
                wt = wqk_pool.tile([128, DCA, 128], BF16, tag="wA", name="wtA")
                nc.sync.dma_start(wt[:], wd[h][:, 0:DCA, :])
            for sc in range(SC512):
                sl = bass.ts(sc, 512)
                ps = ps1.tile([128, 512], F32, tag="ps", name="ps")
                for i in range(DCA):
                    nc.tensor.matmul(
                        ps[:], wt[:, i, :], xa_slice(i, sc * 512, 512),
                        start=(i == 0), stop=(i == DCA - 1),
                    )
                nc.vector.tensor_copy(out=dst[:, sl], in_=ps[:])

        def v_chain_a(s):
            ps = ps1.tile([128, GM], F32, tag="ps", name="ps")
            for i in range(DCA):
                nc.tensor.matmul(
                    ps[:], xa_slice(i, s * 128, 128), wva[:, i, :],
                    start=(i == 0), stop=(i == DCA - 1),
                )
            nc.scalar.copy(out=v_t[s][:], in_=ps[:])

        xtb = []
        wvb_box = []

        def dma_xb(rng, with_wv=False):
            for dc in rng:
                xb = xb_pool.tile([128, SEQ], BF16, tag=f"xb{dc}", name=f"xb{dc}")
                nc.sync.dma_start(xb[:], xt_d[dc])
                xtb.append(xb)
            if with_wv:
                wvb = xb_pool.tile([128, DCB, GM], BF16, tag="wvB", name="wvB")
                nc.sync.dma_start(wvb[:], wv_d[:, DCA:DC, :])
                wvb_box.append(wvb)

        for h in range(G_HEADS):
            qk_chain_a(h, wq_d, q_t[h], wt=wt_first if h == 0 else None)
            qk_chain_a(h, wk_d, k_t[h], wt=wt_k0 if h == 0 else None)
            if h == 1:
                dma_xb(range(DCA, 10))
                for s in range(0, 8):
                    v_chain_a(s)
            if h == 2:
                # rope tables: first needed at the start of pass B
                nc.sync.dma_start(ce_t[:], ce_d)
                nc.sync.dma_start(s2_t[:], s2_d)
            if h == 3:
                dma_xb(range(10, DC), with_wv=True)
                for s in range(8, SC128):
                    v_chain_a(s)

        xa_pool.release()

        # lives from the end of part 1 through part 2
        est_pool = tc.alloc_tile_pool(name="est", bufs=10)
        if with_mask:
            mask_pool = tc.alloc_tile_pool(name="mask", bufs=4)

        e_of = {}

        def emit_st(u, jc2):
            # scores for k-chunk pair (2*jc2, 2*jc2+1) of unit u, then exp
            ic, h = divmod(u, 4)
            isl = bass.ts(ic, 512)
            st = ps_st.tile([128, 1024], F32, tag="st", name="st")
            for half in range(2):
                j = 2 * jc2 + half
                nc.tensor.matmul(
                    st[:, bass.ts(half, 512)],
                    k_t[h][:, bass.ts(j, 128)], q_t[h][:, isl],
                    start=True, stop=True,
                )
            if with_mask:
                mtl = mask_pool.tile([128, 1024], F32, tag="m", name="mtl")
                for half in range(2):
                    j = 2 * jc2 + half
                    nc.sync.dma_start(mtl[:, bass.ts(half, 512)], mt_d[j][:, isl])
                nc.vector.tensor_add(out=st[:], in0=st[:], in1=mtl[:])
            e = est_pool.tile([128, 1024], BF16, tag="e", name="e")
            nc.scalar.activation(
                out=e[:], in_=st[:], func=mybir.ActivationFunctionType.Exp
            )
            e_of[(u, jc2)] = e

        # ---------------- part 1, pass B: dc 4..15 ----------------
        # rope is pipelined one chain behind: emit_tail flushes the shuffle+add
        # of the previous chain so the PE never waits on the DVE/gpsimd line
        rope_pend = []

        def rope_head(dst, sl):
            t1 = rope_pool.tile([128, 512], BF16, tag="t1", name="t1")
            nc.gpsimd.tensor_mul(out=t1[:], in0=dst[:, sl], in1=s2_t[:, sl])
            t3 = rope_pool.tile([128, 512], BF16, tag="t3", name="t3")
            nc.gpsimd.tensor_mul(out=t3[:], in0=dst[:, sl], in1=ce_t[:, sl])
            rope_pend.append((dst, sl, t1, t3))

        def rope_tail():
            while rope_pend:
                dst, sl, t1, t3 = rope_pend.pop(0)
                t2 = rope_pool.tile([128, 512], BF16, tag="t2", name="t2")
                nc.vector.stream_shuffle(t2[:], t1[:], _SWAP_MASK)
                nc.vector.tensor_add(out=dst[:, sl], in0=t2[:], in1=t3[:])

        def qk_chain_b(h, wd, dst):
            wt = wqk_pool.tile([128, DCB, 128], BF16, tag="wB", name="wtB")
            nc.sync.dma_start(wt[:], wd[h][:, DCA:DC, :])
            for sc in range(SC512):
                sl = bass.ts(sc, 512)
                ps = ps1.tile([128, 512], F32, tag="ps", name="ps")
                for i in range(DCB):
                    nc.tensor.matmul(
                        ps[:], wt[:, i, :], xtb[i][:, sl],
                        start=(i == 0), stop=(i == DCB - 1),
                    )
                nc.vector.tensor_add(out=dst[:, sl], in0=ps[:], in1=dst[:, sl])
                rope_head(dst, sl)
                rope_tail_deferred()

        # two-chain-deep deferral: the DVE is in-order, so the rope tail of
        # chain n must be emitted after chain n+2's PSUM add to keep the
        # DVE->gpsimd->DVE dependency line off the chain recurrence
        def rope_tail_deferred():
            while len(rope_pend) > 2:
                dst, sl, t1, t3 = rope_pend.pop(0)
                t2 = rope_pool.tile([128, 512], BF16, tag="t2", name="t2")
                nc.vector.stream_shuffle(t2[:], t1[:], _SWAP_MASK)
                nc.vector.tensor_add(out=dst[:, sl], in0=t2[:], in1=t3[:])

        def v_chain_b(s):
            wvb = wvb_box[0]
            ps = ps1.tile([128, GM], F32, tag="ps", name="ps")
            for i in range(DCB):
                nc.tensor.matmul(
                    ps[:], xtb[i][:, bass.ts(s, 128)], wvb[:, i, :],
                    start=(i == 0), stop=(i == DCB - 1),
                )
            nc.vector.tensor_add(out=v_t[s][:], in0=ps[:], in1=v_t[s][:])

        for h in range(G_HEADS):
            qk_chain_b(h, wq_d, q_t[h])
            qk_chain_b(h, wk_d, k_t[h])
            if h == 1:
                for s in range(0, 8):
                    v_chain_b(s)
            if h == 2:
                for m in range(G_HEADS):
                    nc.sync.dma_start(wo_t[m][:], wo_d[m])
            if h == 3:
                rope_tail()
                # interleave the first 4 score-pairs of unit 0 into the v tail
                # so ACT gets a head start on exp before part 2 begins
                for s in range(8, SC128):
                    v_chain_b(s)
                    if s % 2 == 1:
                        emit_st(0, (s - 9) // 2)
        rope_tail()

        ps1.release()
        rope_pool.release()
        wqk_pool.release()
        xb_pool.release()

        # ---------------- part 2: attention + output projection ----------------
        ps_ap = tc.alloc_tile_pool(name="ps_ap", bufs=2, space="PSUM")
        ps_dn = tc.alloc_tile_pool(name="ps_dn", bufs=1, space="PSUM")
        nrm_pool = tc.alloc_tile_pool(name="nrm", bufs=2)
        fout_pool = tc.alloc_tile_pool(name="fout", bufs=2)

        fout_of = {}

        def p3_chain(pu, nck, small_dma=False):
            pic, s_loc = divmod(pu, 4)
            s_glob = pic * 4 + s_loc
            pp = ps_ap.tile([128, 512], F32, tag="p3", name="pp", bufs=1)
            ssl = bass.ts(s_glob, 128)
            nsl = bass.ts(nck, 512)
            for m in range(G_HEADS):
                nc.tensor.matmul(
                    pp[:], o_t[m][:, ssl], wo_t[m][:, nsl],
                    start=(m == 0), stop=(m == G_HEADS - 1),
                )
            if small_dma:
                f = fout_pool.tile([128, 512], F32, tag="fs", name="fs", bufs=2)
                nc.vector.tensor_copy(out=f[:], in_=pp[:])
                nc.sync.dma_start(out_d[s_glob][:, nsl], f[:])
                return
            if nck == 0:
                fout_of[pu] = fout_pool.tile([128, SEQ], F32, tag="fout", name="fout")
            f = fout_of[pu]
            nc.vector.tensor_copy(out=f[:, nsl], in_=pp[:])
            if nck == SC512 - 1:
                nc.sync.dma_start(out_d[s_glob], f[:])
                del fout_of[pu]

        def norm(u):
            ic, h = divmod(u, 4)
            isl = bass.ts(ic, 512)
            den, acc = den_acc[u]
            rsb = nrm_pool.tile([128, 4], F32, tag="rsb", name="rsb")
            nc.vector.reciprocal_approx_fast(out=rsb[:], in_=den[:])
            bc = nrm_pool.tile([128, 512], F32, tag="bc", name="bc")
            for qs in range(4):
                rr = nrm_pool.tile([1, 128], F32, tag=f"rr{qs}", name="rr")
                nc.sync.dma_start(rr[:], rsb[:, qs : qs + 1])
                nc.gpsimd.partition_broadcast(bc[:, bass.ts(qs, 128)], rr[:])
            nc.vector.tensor_mul(out=o_t[h][:, isl], in0=acc[:], in1=bc[:])

        den_acc = {}
        # output-projection chains for unit pu run during stream pu+2 (norm of
        # pu has a full stream to drain); the last two units' chains are
        # interleaved into stream 15 / the tail
        for u in range(NU):
            ic, h = divmod(u, 4)
            hsl = bass.ts(h, 128)
            acc = ps_ap.tile([128, 512], F32, tag="acc", name="acc")
            den = ps_dn.tile([128, 4], F32, tag="den", name="den")
            den_acc[u] = (den, acc)
            last_u = u == NU - 1
            for jc2 in range(NJ2):
                if jc2 < 4:
                    emit_st(u, jc2 + 4)
                elif not last_u:
                    emit_st(u + 1, jc2 - 4)
                e = e_of.pop((u, jc2))
                for half in range(2):
                    j = 2 * jc2 + half
                    esl = e[:, bass.ts(half, 512)]
                    nc.tensor.matmul(
                        acc[:], v_t[j][:, hsl], esl,
                        start=(j == 0), stop=(j == SC128 - 1),
                    )
                    # one PSUM accumulation group for the whole [128,4] tile:
                    # the 2KB zero-region auto-starts each byte on first touch
                    for qs in range(4):
                        nc.tensor.matmul(
                            den[:, qs : qs + 1],
                            e[:, half * 512 + qs * 128 : half * 512 + (qs + 1) * 128],
                            ones_t[:],
                            start=(j == 0 and qs == 0),
                            stop=(j == SC128 - 1 and qs == 3),
                        )
                if jc2 == NJ2 - 1:
                    norm(u)
                if jc2 % 2 == 1 and u >= 2:
                    p3_chain(u - 2, jc2 // 2)
                if last_u and jc2 % 2 == 0:
                    # unit 14's chains fill the st slots stream 15 lacks
                    p3_chain(NU - 2, jc2 // 2, small_dma=True)

        # tail: output projection for the last unit
        for nck in range(SC512):
            p3_chain(NU - 1, nck, small_dma=True)

        if dbg:
            nc.sync.dma_start(dbg_d[0], q_t[0][:])
            nc.sync.dma_start(dbg_d[1], k_t[0][:])
            nc.sync.dma_start(dbg_d[2], o_t[0][:])
            nc.sync.dma_start(dbg_d[3][:, 0:GM], v_t[0][:])

        ps_dn.release()
        ps_ap.release()
        ps_st.release()
        fout_pool.release()
        nrm_pool.release()
        if with_mask:
            mask_pool.release()
        est_pool.release()
        persist.release()

    nc.compile()
    return nc


_CACHE = {}


def _get_nc(with_mask: bool):
    if with_mask not in _CACHE:
        _CACHE[with_mask] = build(with_mask)
    return _CACHE[with_mask]


def kernel(in_token, freqs_cos, freqs_sin, mask, wq, wk, wv, wo):
    return _run(in_token, freqs_cos, freqs_sin, mask, wq, wk, wv, wo)


def run_traced(in_token, freqs_cos, freqs_sin, mask, wq, wk, wv, wo):
    """Test-only: run with NTFF tracing, return (output, BassKernelResults)."""
    return _run(in_token, freqs_cos, freqs_sin, mask, wq, wk, wv, wo, trace=True)


def _run(in_token, freqs_cos, freqs_sin, mask, wq, wk, wv, wo, trace=False):
    in_token = np.asarray(in_token, dtype=np.float32)
    freqs_cos = np.asarray(freqs_cos, dtype=np.float32)
    freqs_sin = np.asarray(freqs_sin, dtype=np.float32)
    mask = np.asarray(mask, dtype=np.float32)
    wq = np.asarray(wq, dtype=np.float32)
    wk = np.asarray(wk, dtype=np.float32)
    wv = np.asarray(wv, dtype=np.float32)
    wo = np.asarray(wo, dtype=np.float32)

    with_mask = bool(np.any(mask))
    nc = _get_nc(with_mask)

    # rope tables in (head_dim, seq) pair-expanded layout, signs/swap baked in
    ce = np.repeat(freqs_cos.T, 2, axis=0).astype(NPBF)  # (128, S)
    s2 = np.empty((HEAD_DIM, SEQ), np.float32)
    s2[0::2] = freqs_sin.T   # even rows: +sin (lands on odd out after swap)
    s2[1::2] = -freqs_sin.T  # odd rows: -sin (lands on even out after swap)
    s2 = s2.astype(NPBF)
    ones = np.ones((128, 1), NPBF)
    if with_mask:
        mt = np.ascontiguousarray(mask.T).reshape(SC128, 128, SEQ)

    xts = [
        np.ascontiguousarray(in_token[b].T).astype(NPBF).reshape(DC, 128, SEQ)
        for b in range(BATCH)
    ]
    # per-head-group weight layouts (shared across the two batch cores)
    gmaps = []
    for g in range(G_HEADS):
        rows = slice(g * GM, (g + 1) * GM)
        # wt[h, p, dc, m] = w[g*512 + h*128 + m, dc*128 + p]
        wqt = np.ascontiguousarray(
            (wq[rows] * INV_SQRT_HD).reshape(G_HEADS, 128, DC, 128).transpose(0, 3, 2, 1)
        ).astype(NPBF)
        wkt = np.ascontiguousarray(
            wk[rows].reshape(G_HEADS, 128, DC, 128).transpose(0, 3, 2, 1)
        ).astype(NPBF)
        # wvt[p, dc, n] = wv[g*512 + n, dc*128 + p]
        wvt = np.ascontiguousarray(
            wv[rows].reshape(GM, DC, 128).transpose(2, 1, 0)
        ).astype(NPBF)
        # wot[mc, hd, n] = wo[n, g*512 + mc*128 + hd]
        wot = np.ascontiguousarray(wo[:, rows].T).astype(NPBF).reshape(G_HEADS, 128, SEQ)
        gmaps.append({"wq": wqt, "wk": wkt, "wv": wvt, "wo": wot})

    in_maps = []
    for b in range(BATCH):
        for g in range(G_HEADS):
            m = {
                "xt": xts[b], "ce": ce, "s2": s2, "ones": ones, **gmaps[g],
            }
            if with_mask:
                m["mt"] = mt
            in_maps.append(m)

    res = run_bass_kernel_spmd(nc, in_maps, core_ids=list(range(8)), trace=trace)

    out = np.zeros((BATCH, SEQ, DIM), np.float32)
    for b in range(BATCH):
        acc = None
        for g in range(G_HEADS):
            p = res.results[b * G_HEADS + g]["out"].reshape(SEQ, DIM)
            acc = p if acc is None else acc + p
        out[b] = acc
    if trace:
        return out, res
    return out
